# revision 1
# baseline (speedup 1.0000x reference)
"""Trainium2 Bass kernel for nn_AlignCriterion (align loss).

Strategy: pure data-parallel over batch (8 batches per core, 8 cores).
The O(B*N^2*C) correlation/assignment einsums are algebraically collapsed:

  gclc_cor_loss = -0.15 * sum_b [ T1_b - T2_b ]
    T1_b = sum_{q,c} P[q,c] * R[q,c]
      P[q,c] = sum_n w~g[n,q] * gn[n,c]     (gn = normalized gc, w~ = masked softmax)
      R[q,c] = sum_m w~l[m,q] * ln[m,c]
    T2_b = sum_q (alpha'_q + (0.1 - g) * beta_q) * v_q
      alpha'_q = sum_n w~g[n,q] * r_n,  r_n = (gn[n,:] . s_lc)/784
      beta_q   = sum_n w~g[n,q],  v_q = sum_m w~l[m,q]
      g = sum_b (s_gc . s_lc) / (B*N*M)   (global scalar, combined on host)

  query CE loss computed per-batch from z = [q0n; q1n], sim = z z^T.

Per-core device output: [8, 128, 8] f32 partial sums; host combines in f64.
"""

import sys

import numpy as np

sys.path.insert(0, "/opt/trn_rl_repo")

import concourse.bass as bass  # noqa: E402,F401
import concourse.mybir as mybir  # noqa: E402
import concourse.tile as tile  # noqa: E402
from concourse import bacc  # noqa: E402
from concourse.bass_utils import run_bass_kernel_spmd  # noqa: E402
from concourse.masks import make_identity  # noqa: E402

F32 = mybir.dt.float32
BF16 = mybir.dt.bfloat16
I32 = mybir.dt.int32
AF = mybir.ActivationFunctionType
ALU = mybir.AluOpType
AX = mybir.AxisListType

B = 64
N = 784          # 28*28 spatial positions
C = 384
Q = 5
NCORES = 8
BL = B // NCORES  # batches per core = 8
NT = 7           # row tiles per batch: 6 full 128 + 1 tail of 16
TAIL = N - 6 * 128  # 16
NK = 3           # c chunks of 128

_CACHED = {}


def _build():
    nc = bacc.Bacc("TRN2", target_bir_lowering=False, debug=False,
                   num_devices=NCORES)

    gc_in = nc.dram_tensor("gc_in", [BL, N, C], F32, kind="ExternalInput").ap()
    lc_in = nc.dram_tensor("lc_in", [BL, N, C], F32, kind="ExternalInput").ap()
    q0_in = nc.dram_tensor("q0_in", [BL, Q, C], F32, kind="ExternalInput").ap()
    q1_in = nc.dram_tensor("q1_in", [BL, Q, C], F32, kind="ExternalInput").ap()
    att_in = nc.dram_tensor("att_in", [2 * BL, N], I32, kind="ExternalInput").ap()
    out = nc.dram_tensor("out", [BL, 128, 8], F32, kind="ExternalOutput").ap()

    with tile.TileContext(nc) as tc:
        _kernel(tc, out, gc_in, lc_in, q0_in, q1_in, att_in)

    # the installed walrus birverifier rejects EVENT_SEMAPHORE_RANGE_CLEAR
    # (opcode 176, emitted by the Tile kernel-tail sem cleanup). NRT re-inits
    # semaphores per execution, so drop the tail clear entirely.
    for fn in nc.m.functions:
        for blk in fn.blocks:
            il = blk.instructions
            for i in range(len(il) - 1, -1, -1):
                if isinstance(il[i], mybir.InstISA) and il[i].isa_opcode == 176:
                    del il[i]

    nc.compile()
    return nc


def _kernel(tc, out, gc_in, lc_in, q0_in, q1_in, att_in):
    from contextlib import ExitStack
    with ExitStack() as ctx:
        _kernel_inner(ctx, tc, out, gc_in, lc_in, q0_in, q1_in, att_in)


def _kernel_inner(ctx, tc, out, gc_in, lc_in, q0_in, q1_in, att_in):
    nc = tc.nc
    E = float(np.exp(1.0))

    consts = ctx.enter_context(tc.tile_pool(name="consts", bufs=1))
    sb = ctx.enter_context(tc.tile_pool(name="sb", bufs=2))
    sbs = ctx.enter_context(tc.tile_pool(name="sbs", bufs=3))
    ps_t = ctx.enter_context(tc.tile_pool(name="ps_t", bufs=2, space="PSUM"))
    ps_asg = ctx.enter_context(tc.tile_pool(name="ps_asg", bufs=2, space="PSUM"))
    ps_misc = ctx.enter_context(tc.tile_pool(name="ps_misc", bufs=2, space="PSUM"))
    ps_init = ctx.enter_context(tc.tile_pool(name="ps_init", bufs=1, space="PSUM"))

    # ---- constants ----
    ident_bf = consts.tile([128, 128], BF16, tag="ident_bf")
    make_identity(nc, ident_bf[:])
    ident_f = consts.tile([16, 16], F32, tag="ident_f")
    make_identity(nc, ident_f[:])
    # partner mask [10,10]: mask[r, (r+Q)%2Q] = 1
    pmask = consts.tile([10, 10], F32, tag="pmask")
    nc.gpsimd.memset(pmask[:], 0.0)
    nc.gpsimd.affine_select(out=pmask[:], in_=pmask[:], compare_op=ALU.not_equal,
                            fill=1.0, base=-Q, pattern=[[-1, 10]], channel_multiplier=1)
    nc.gpsimd.affine_select(out=pmask[:], in_=pmask[:], compare_op=ALU.not_equal,
                            fill=1.0, base=Q, pattern=[[-1, 10]], channel_multiplier=1)

    # ---- masks: att [16, 784] i32 -> U [128, 7, 16] f32 (col j = crop j) ----
    att_i = consts.tile([2 * BL, N], I32, tag="att_i")
    nc.sync.dma_start(att_i[:], att_in[:, :])
    att_f = consts.tile([2 * BL, N], F32, tag="att_f")
    nc.vector.tensor_copy(att_f[:], att_i[:])
    U = consts.tile([128, NT, 2 * BL], F32, tag="U")
    for t in range(NT):
        w = 128 if t < 6 else TAIL
        pt = ps_init.tile([128, 16], F32, tag="upsum")
        nc.tensor.transpose(pt[:w, :], att_f[:, t * 128:t * 128 + w], ident_f[:, :])
        nc.scalar.copy(U[:w, t, :], pt[:w, :])

    for b in range(BL):
        _batch(tc, b, out, gc_in, lc_in, q0_in, q1_in,
               sb, sbs, ps_t, ps_asg, ps_misc, ident_bf, pmask, U, E)


def _batch(tc, b, out, gc_in, lc_in, q0_in, q1_in,
           sb, sbs, ps_t, ps_asg, ps_misc, ident_bf, pmask, U, E):
    nc = tc.nc

    # ---------- load ----------
    gc_nat = sb.tile([128, NT, C], F32, tag="gc_nat")
    lc_nat = sb.tile([128, NT, C], F32, tag="lc_nat")
    for src, dst in ((gc_in, gc_nat), (lc_in, lc_nat)):
        nc.sync.dma_start(dst[:, 0:6, :],
                          src[b, 0:768, :].rearrange("(t p) c -> p t c", p=128))
        nc.sync.dma_start(dst[0:TAIL, 6, :], src[b, 768:N, :])

    q0 = sbs.tile([Q, C], F32, tag="q0")
    q1 = sbs.tile([Q, C], F32, tag="q1")
    nc.sync.dma_start(q0[:], q0_in[b])
    nc.sync.dma_start(q1[:], q1_in[b])

    # ---------- row stats: ss = sum_c x^2 ; inv = 1/max(sqrt(ss),1e-10) ----------
    ss = sbs.tile([128, 2 * NT + 2], F32, tag="ss")
    sq_scr = sbs.tile([128, C], BF16, tag="sq_scr")
    for t in range(NT):  # ACT Square with fused row-accumulate
        nc.scalar.activation(sq_scr[:], gc_nat[:, t, :], AF.Square,
                             accum_out=ss[:, t:t + 1])
    sq2 = sbs.tile([128, C], BF16, tag="sq2")
    for t in range(NT):
        nc.scalar.activation(sq2[:], lc_nat[:, t, :], AF.Square,
                             accum_out=ss[:, NT + t:NT + t + 1])
    qs_scr = sbs.tile([Q, C], BF16, tag="qs_scr")
    nc.scalar.activation(qs_scr[:], q0[:], AF.Square,
                         accum_out=ss[:Q, 2 * NT:2 * NT + 1])
    nc.scalar.activation(qs_scr[:], q1[:], AF.Square,
                         accum_out=ss[:Q, 2 * NT + 1:2 * NT + 2])

    inv = sbs.tile([128, 2 * NT + 2], F32, tag="inv")
    nc.scalar.sqrt(inv[:], ss[:])
    nc.vector.tensor_scalar_max(inv[:], inv[:], 1e-10)
    nc.vector.reciprocal(inv[:], inv[:])

    # ---------- normalized bf16 copies ----------
    gn = sb.tile([128, NT, C], BF16, tag="gn")
    ln = sb.tile([128, NT, C], BF16, tag="ln")
    for t in range(NT):
        nc.vector.tensor_scalar_mul(gn[:, t, :], gc_nat[:, t, :], inv[:, t:t + 1])
        nc.vector.tensor_scalar_mul(ln[:, t, :], lc_nat[:, t, :],
                                    inv[:, NT + t:NT + t + 1])
    q0n = sbs.tile([Q, C], BF16, tag="q0n")
    q1n = sbs.tile([Q, C], BF16, tag="q1n")
    nc.vector.tensor_scalar_mul(q0n[:], q0[:], inv[:Q, 2 * NT:2 * NT + 1])
    nc.vector.tensor_scalar_mul(q1n[:], q1[:], inv[:Q, 2 * NT + 1:2 * NT + 2])

    # ---------- zstag [128, 3, 16] bf16: [0:5]=q0nT, [5]=slcN, [6:11]=q0nT, [11:16]=q1nT ----------
    zstag = sbs.tile([128, NK, 16], BF16, tag="zstag")
    for k in range(NK):
        pt = ps_t.tile([128, 4, 128], BF16, tag="tpsum")
        nc.tensor.transpose(pt[:, 0, 0:Q], q0n[:, k * 128:(k + 1) * 128],
                            ident_bf[:Q, :Q])
        nc.tensor.transpose(pt[:, 1, 0:Q], q1n[:, k * 128:(k + 1) * 128],
                            ident_bf[:Q, :Q])
        nc.scalar.copy(zstag[:, k, 0:Q], pt[:, 0, 0:Q])
        nc.scalar.copy(zstag[:, k, 6:6 + Q], pt[:, 0, 0:Q])
        nc.scalar.copy(zstag[:, k, 11:16], pt[:, 1, 0:Q])

    # misc psum: pr [128,2,3,8] at cols 0:48, abv [16,4] at 48:52, sim [10,10] at 52:62
    misc = ps_misc.tile([128, 64], F32, tag="misc")
    pr = misc[:, 0:48].rearrange("p (i k e) -> p i k e", i=2, k=NK)
    abv = misc[:16, 48:52]
    sim = misc[:10, 52:62]
    rn_ones = sbs.tile([128, NT, 2], BF16, tag="rn_ones")
    nc.gpsimd.memset(rn_ones[:], 1.0)

    # ---------- per-tensor phase: lc first (produces slcN), then gc ----------
    for side in ("lc", "gc"):
        xn = ln if side == "lc" else gn
        crop = (BL + b) if side == "lc" else b
        pri = 0 if side == "gc" else 1

        # -- transpose xn -> staging [128c, 3k, 784n] bf16 --
        stag = sb.tile([128, NK, N], BF16, tag=f"stag_{side}")
        for k in range(NK):
            for half, (t0, nth) in enumerate(((0, 4), (4, 3))):
                pt = ps_t.tile([128, 4, 128], BF16, tag="tpsum")
                for j in range(nth):
                    t = t0 + j
                    w = 128 if t < 6 else TAIL
                    nc.tensor.transpose(pt[:, j, 0:w],
                                        xn[0:w, t, k * 128:(k + 1) * 128],
                                        ident_bf[0:w, 0:w])
                eng = nc.vector if (k + half) % 2 == 0 else nc.scalar
                copy = (eng.tensor_copy if eng is nc.vector else eng.copy)
                if t0 + nth <= 6:
                    copy(stag[:, k, t0 * 128:(t0 + nth) * 128].rearrange(
                        "p (j w) -> p j w", j=nth), pt[:, 0:nth, :])
                else:
                    copy(stag[:, k, t0 * 128:(t0 + nth - 1) * 128].rearrange(
                        "p (j w) -> p j w", j=nth - 1), pt[:, 0:nth - 1, :])
                    copy(stag[:, k, 6 * 128:6 * 128 + TAIL],
                         pt[:, nth - 1, 0:TAIL])

        # -- assignment logits: asg [128, 7, 8] f32; gc also gets r col 5 --
        ncol = 6 if side == "gc" else Q
        rcols = slice(0, 6) if side == "gc" else slice(11, 16)
        asg = ps_asg.tile([128, NT, 8], F32, tag="asg")
        for t in range(NT):
            w = 128 if t < 6 else TAIL
            for k in range(NK):
                nc.tensor.matmul(asg[0:w, t, 0:ncol],
                                 stag[:, k, t * 128:t * 128 + w],
                                 zstag[:, k, rcols],
                                 start=(k == 0), stop=(k == NK - 1))

        # -- softmax over Q with relu; w~ = e * (u/sumexp) --
        e_t = sbs.tile([128, NT, Q], F32, tag=f"e_{side}")
        nc.vector.tensor_scalar_max(e_t[:], asg[:, :, 0:Q], 0.0)
        nc.scalar.activation(e_t[:], e_t[:], AF.Exp)
        sume = sbs.tile([128, NT], F32, tag=f"sume_{side}")
        nc.vector.tensor_reduce(sume[:], e_t[:], axis=AX.X, op=ALU.add)
        nc.vector.reciprocal(sume[:], sume[:])
        stil = sbs.tile([128, NT], F32, tag=f"stil_{side}")
        nc.vector.tensor_tensor(out=stil[:], in0=sume[:], in1=U[:, :, crop],
                                op=ALU.mult)
        wt = sbs.tile([128, NT, 6], BF16, tag=f"wt_{side}")
        nc.gpsimd.memset(wt[:, :, Q:6], 1.0 if side == "gc" else 1.0 / N)
        for t in range(NT):
            nc.vector.tensor_scalar_mul(wt[:, t, 0:Q], e_t[:, t, :],
                                        stil[:, t:t + 1])

        # -- P/R (+ s col): accumulate over tiles --
        for k in range(NK):
            for t in range(NT):
                w = 128 if t < 6 else TAIL
                nc.tensor.matmul(pr[:, pri, k, 0:6],
                                 xn[0:w, t, k * 128:(k + 1) * 128],
                                 wt[0:w, t, :],
                                 start=(t == 0), stop=(t == NT - 1))

        if side == "lc":
            for k in range(NK):  # slcN -> zstag col 5 (bf16)
                nc.scalar.copy(zstag[:, k, Q:Q + 1], pr[:, 1, k, 5:6])
            for t in range(NT):  # vq
                w = 128 if t < 6 else TAIL
                nc.tensor.matmul(abv[:Q, 2:3], wt[0:w, t, 0:Q],
                                 rn_ones[0:w, t, 0:1],
                                 start=(t == 0), stop=(t == NT - 1))
        else:
            nc.vector.tensor_copy(rn_ones[:, :, 0], asg[:, :, 5])
            for t in range(NT):  # alpha', beta
                w = 128 if t < 6 else TAIL
                nc.tensor.matmul(abv[:Q, 0:2], wt[0:w, t, 0:Q],
                                 rn_ones[0:w, t, :],
                                 start=(t == 0), stop=(t == NT - 1))

    # ---------- query CE ----------
    for k in range(NK):
        nc.tensor.matmul(sim[:, :], zstag[:, k, 6:16], zstag[:, k, 6:16],
                         start=(k == 0), stop=(k == NK - 1))
    esum = sbs.tile([10, 4], F32, tag="esum")
    esim = sbs.tile([10, 10], F32, tag="esim")
    nc.scalar.activation(esim[:], sim[:, :], AF.Exp, accum_out=esum[:, 0:1])
    nc.vector.tensor_scalar_add(esum[:, 1:2], esum[:, 0:1], -E)
    nc.scalar.activation(esum[:, 2:3], esum[:, 1:2], AF.Ln)
    pos_scr = sbs.tile([10, 10], F32, tag="pos_scr")
    nc.vector.tensor_tensor(out=pos_scr[:], in0=sim[:, :], in1=pmask[:],
                            op=ALU.mult)
    nc.vector.tensor_reduce(esum[:, 3:4], pos_scr[:], axis=AX.X, op=ALU.add)
    ce = sbs.tile([10, 1], F32, tag="ce")
    nc.vector.tensor_tensor(out=ce[:], in0=esum[:, 2:3], in1=esum[:, 3:4],
                            op=ALU.subtract)

    # ---------- batch partials -> out[b] ----------
    ot = sbs.tile([128, 8], F32, tag="ot")
    nc.gpsimd.memset(ot[:], 0.0)
    # TensorTensor may read only one input from PSUM: drain P (gc side) to SBUF
    psb = sbs.tile([128, NK, 6], F32, tag="psb")
    nc.scalar.copy(psb[:], pr[:, 0, :, 0:6])
    t1_scr = sbs.tile([128, NK, Q], F32, tag="t1_scr")
    nc.vector.tensor_tensor(out=t1_scr[:], in0=psb[:, :, 0:Q],
                            in1=pr[:, 1, :, 0:Q], op=ALU.mult)
    nc.vector.tensor_reduce(ot[:, 0:1], t1_scr[:], axis=AX.XY, op=ALU.add)
    g_scr = sbs.tile([128, NK], F32, tag="g_scr")
    nc.vector.tensor_tensor(out=g_scr[:], in0=psb[:, :, 5],
                            in1=pr[:, 1, :, 5], op=ALU.mult)
    nc.vector.tensor_reduce(ot[:, 1:2], g_scr[:], axis=AX.X, op=ALU.add)
    nc.scalar.copy(ot[:10, 2:3], ce[:])
    nc.scalar.copy(ot[:Q, 3:6], abv[:Q, 0:3])
    nc.sync.dma_start(out[b], ot[:])


def _combine(results):
    T1 = 0.0
    G = 0.0
    ce_sum = 0.0
    abv = []
    for r in results:
        o = np.asarray(r["out"], dtype=np.float64)  # [BL, 128, 8]
        T1 += o[:, :, 0].sum()
        G += o[:, :, 1].sum()
        ce_sum += o[:, :10, 2].sum()
        abv.append(o[:, :Q, 3:6])
    abv = np.concatenate(abv, 0)  # [B, Q, 3] : alpha', beta, vq
    g = G / (B * N)
    T2 = ((abv[:, :, 0] + (0.1 - g) * abv[:, :, 1]) * abv[:, :, 2]).sum()
    loss1 = -0.15 * (T1 - T2)
    loss2 = ce_sum / (B * 2 * Q)
    return np.float32(loss1 + loss2)


def kernel(all_queries_0, all_queries_1, gc_output, lc_output, attn_hard,
           gc_spatial_res=None, lc_spatial_res=None):
    if "nc" not in _CACHED:
        _CACHED["nc"] = _build()
    nc = _CACHED["nc"]

    gc = np.ascontiguousarray(np.asarray(gc_output, dtype=np.float32))
    lc = np.ascontiguousarray(np.asarray(lc_output, dtype=np.float32)[:, 0])
    q0 = np.ascontiguousarray(np.asarray(all_queries_0, dtype=np.float32))
    q1 = np.ascontiguousarray(np.asarray(all_queries_1, dtype=np.float32))
    att = np.asarray(attn_hard, dtype=np.int32).reshape(2 * B, N)

    in_maps = []
    for i in range(NCORES):
        s = slice(i * BL, (i + 1) * BL)
        in_maps.append({
            "gc_in": gc[s],
            "lc_in": lc[s],
            "q0_in": q0[s],
            "q1_in": q1[s],
            "att_in": np.ascontiguousarray(
                np.concatenate([att[s], att[B + i * BL:B + (i + 1) * BL]], 0)),
        })
    res = run_bass_kernel_spmd(nc, in_maps, core_ids=list(range(NCORES)))
    return _combine(res.results)



# revision 8
# speedup vs baseline: 1.9553x; 1.9553x over previous
"""Trainium2 Bass kernel for nn_AlignCriterion (align loss).

Data-parallel over batch: 8 batches per core, 8 cores. The O(B*N^2*C)
correlation/assignment einsums are algebraically collapsed (see _combine).

Per batch the device computes, with gc/lc shipped in TWO layouts
(natural [n, c] bf16 with a ones column, and transposed [c, n] fp8):

  sim   = z_raw z_raw^T                  (bf16, for CE + query norms)
  asgT  = q_raw^T @ xT_raw               (fp8 streams, [10, 800] logits)
  softmax weights from relu(asg * inv_q * inv_n), masked by attn
  P/R   = wt^T @ x_nat                   (bf16, [11, 385] incl. beta col)

The weight matrix wt packs [wg*inv | inv | wg] columns so one matmul
yields P (inv-folded), s_gc/s_lc (inv column), and beta/v (raw wg
against the natural layout's ones column). Host combines in f64.
"""

import sys

import numpy as np

sys.path.insert(0, "/opt/trn_rl_repo")

import ml_dtypes  # noqa: E402
import concourse.bass as bass  # noqa: E402,F401
import concourse.mybir as mybir  # noqa: E402
import concourse.tile as tile  # noqa: E402
from concourse import bacc  # noqa: E402
from concourse.bass_utils import run_bass_kernel_spmd  # noqa: E402
from concourse.masks import make_identity  # noqa: E402

F32 = mybir.dt.float32
BF16 = mybir.dt.bfloat16
FP8 = mybir.dt.float8e4
AF = mybir.ActivationFunctionType
ALU = mybir.AluOpType
AX = mybir.AxisListType

BF = ml_dtypes.bfloat16
F8 = ml_dtypes.float8_e4m3

B = 64
N = 784          # 28*28 spatial positions
C = 384
Q = 5
NCORES = 8
BL = B // NCORES  # batches per core = 8
NT = 7           # n tiles of 128 (784 padded to 896)
NK = 3           # c chunks of 128
NP = 800         # padded n for the transposed layout
H = NP // 2      # psum half width

_CACHED = {}


def _build():
    nc = bacc.Bacc("TRN2", target_bir_lowering=False, debug=False,
                   num_devices=NCORES)

    natg = nc.dram_tensor("natg", [BL, 128, NT, C + 1], BF16, kind="ExternalInput").ap()
    natl = nc.dram_tensor("natl", [BL, 128, NT, C + 1], BF16, kind="ExternalInput").ap()
    trag = nc.dram_tensor("trag", [BL, 128, NK, NP], FP8, kind="ExternalInput").ap()
    tral = nc.dram_tensor("tral", [BL, 128, NK, NP], FP8, kind="ExternalInput").ap()
    ztq = nc.dram_tensor("ztq", [BL, 128, NK, 16], FP8, kind="ExternalInput").ap()
    ztb = nc.dram_tensor("ztb", [BL, 128, NK, 16], BF16, kind="ExternalInput").ap()
    u_in = nc.dram_tensor("u_in", [128, BL, 2, NT], F32, kind="ExternalInput").ap()
    out = nc.dram_tensor("out", [BL, 16, 780], F32, kind="ExternalOutput").ap()

    with tile.TileContext(nc) as tc:
        _kernel(tc, out, natg, natl, trag, tral, ztq, ztb, u_in)

    # the installed walrus birverifier rejects EVENT_SEMAPHORE_RANGE_CLEAR
    # (opcode 176, emitted by the Tile kernel-tail sem cleanup). NRT re-inits
    # semaphores per execution, so drop the tail clear entirely.
    for fn in nc.m.functions:
        for blk in fn.blocks:
            il = blk.instructions
            for i in range(len(il) - 1, -1, -1):
                if isinstance(il[i], mybir.InstISA) and il[i].isa_opcode == 176:
                    del il[i]

    nc.compile()
    return nc


def _kernel(tc, out, natg, natl, trag, tral, ztq, ztb, u_in):
    from contextlib import ExitStack
    with ExitStack() as ctx:
        _kernel_inner(ctx, tc, out, natg, natl, trag, tral, ztq, ztb, u_in)


def _kernel_inner(ctx, tc, out, natg, natl, trag, tral, ztq, ztb, u_in):
    nc = tc.nc

    consts = ctx.enter_context(tc.tile_pool(name="consts", bufs=1))
    sbin = ctx.enter_context(tc.tile_pool(name="sbin", bufs=3))
    sbq = ctx.enter_context(tc.tile_pool(name="sbq", bufs=2))
    sbs = ctx.enter_context(tc.tile_pool(name="sbs", bufs=2))
    ps_ag = ctx.enter_context(tc.tile_pool(name="ps_ag", bufs=1, space="PSUM"))
    ps_al = ctx.enter_context(tc.tile_pool(name="ps_al", bufs=1, space="PSUM"))
    ps_tp = ctx.enter_context(tc.tile_pool(name="ps_tp", bufs=1, space="PSUM"))
    ps_sim = ctx.enter_context(tc.tile_pool(name="ps_sim", bufs=1, space="PSUM"))
    ps_p = ctx.enter_context(tc.tile_pool(name="ps_p", bufs=1, space="PSUM"))
    ps_r = ctx.enter_context(tc.tile_pool(name="ps_r", bufs=1, space="PSUM"))

    ident_bf = consts.tile([48, 48], BF16, tag="ident_bf")
    make_identity(nc, ident_bf[:])
    ident_f = consts.tile([16, 16], F32, tag="ident_f")
    make_identity(nc, ident_f[:])
    U = consts.tile([128, BL, 2, NT], F32, tag="U")
    nc.sync.dma_start(U[:], u_in[:, :, :, :])

    for b in range(BL):
        # ---------- loads ----------
        ng = sbin.tile([128, NT, C + 1], BF16, tag="ng")
        nl = sbin.tile([128, NT, C + 1], BF16, tag="nl")
        tg = sbin.tile([128, NK, NP], FP8, tag="tg")
        tl = sbin.tile([128, NK, NP], FP8, tag="tl")
        nc.sync.dma_start(ng[:], natg[b])
        nc.sync.dma_start(nl[:], natl[b])
        nc.sync.dma_start(tg[:], trag[b])
        nc.sync.dma_start(tl[:], tral[b])
        zq = sbq.tile([128, NK, 16], FP8, tag="zq")
        zb = sbq.tile([128, NK, 16], BF16, tag="zb")
        nc.sync.dma_start(zq[:], ztq[b])
        nc.sync.dma_start(zb[:], ztb[b])

        out_sb = sbs.tile([16, 780], F32, tag="out_sb")
        nc.gpsimd.memset(out_sb[:], 0.0)

        # ---------- sim (CE) + query inverse norms ----------
        sim_ps = ps_sim.tile([48, 12], F32, tag="sim_ps")
        for k in range(NK):
            nc.tensor.matmul(sim_ps[0:10, 0:10], zb[:, k, 0:10], zb[:, k, 0:10],
                             start=(k == 0), stop=(k == NK - 1))
        dg_scr = sbs.tile([16, 10], F32, tag="dg_scr")
        invq = sbs.tile([48, 4], F32, tag="invq")
        nc.vector.tensor_tensor(out=dg_scr[0:10, :], in0=ident_f[0:10, 0:10],
                                in1=sim_ps[0:10, 0:10], op=ALU.mult)
        nc.vector.tensor_reduce(invq[0:10, 0:1], dg_scr[0:10, :],
                                axis=AX.X, op=ALU.add)
        nc.scalar.sqrt(invq[0:10, 1:2], invq[0:10, 0:1])
        nc.vector.tensor_scalar_max(invq[0:10, 2:3], invq[0:10, 1:2], 1e-10)
        nc.vector.reciprocal(invq[0:10, 3:4], invq[0:10, 2:3])
        # shift q1 inv-norms (rows 5:10) to partitions 32:37 via PE
        nc.tensor.matmul(sim_ps[32:37, 10:11], ident_f[0:10, 5:10],
                         invq[0:10, 3:4], start=True, stop=True,
                         tile_position=(0, 32))
        nc.scalar.copy(invq[32:37, 0:1], sim_ps[32:37, 10:11])
        nc.scalar.copy(out_sb[0:10, 770:780], sim_ps[0:10, 0:10])

        # ---------- row inverse norms (ss over c, per n) ----------
        sq_g = sbs.tile([128, NT, C], BF16, tag="sq_g")
        sq_l = sbs.tile([128, NT, C], BF16, tag="sq_l")
        nc.scalar.activation(sq_g[:], ng[:, :, 0:C], AF.Square)
        nc.vector.tensor_tensor(out=sq_l[:], in0=nl[:, :, 0:C],
                                in1=nl[:, :, 0:C], op=ALU.mult)
        ss = sbs.tile([128, 2 * NT], F32, tag="ss")
        nc.vector.tensor_reduce(ss[:, 0:NT], sq_g[:], axis=AX.X, op=ALU.add)
        nc.vector.tensor_reduce(ss[:, NT:2 * NT], sq_l[:], axis=AX.X, op=ALU.add)
        rt = sbs.tile([128, 2 * NT], F32, tag="rt")
        nc.scalar.sqrt(rt[:], ss[:])
        nc.vector.tensor_scalar_max(rt[:], rt[:], 1e-10)
        inv = sbs.tile([128, 2 * NT], F32, tag="inv")
        nc.vector.reciprocal(inv[:], rt[:])

        # ---------- assignment logits (fp8): gc rows 0:5, lc rows 32:37 ----------
        ag_ps = ps_ag.tile([16, 2, H], F32, tag="ag_ps", padded_shape=[16, 2, 512])
        al_ps = ps_al.tile([48, 2, H], F32, tag="al_ps", padded_shape=[48, 2, 512])
        for h in range(2):
            for k in range(NK):
                nc.tensor.matmul(ag_ps[0:5, h, :], zq[:, k, 0:Q],
                                 tg[:, k, H * h:H * (h + 1)],
                                 start=(k == 0), stop=(k == NK - 1))
        for h in range(2):
            for k in range(NK):
                nc.tensor.matmul(al_ps[32:37, h, :], zq[:, k, Q:10],
                                 tl[:, k, H * h:H * (h + 1)],
                                 start=(k == 0), stop=(k == NK - 1),
                                 tile_position=(0, 32))

        # relu(asg * invq): drain to bf16 (gc rows 0:5, lc rows 32:37)
        asgT = sbs.tile([48, NP], BF16, tag="asgT")
        nc.gpsimd.memset(asgT[:], 0.0)
        nc.scalar.activation(asgT[0:5, :].rearrange("p (h n) -> p h n", h=2),
                             ag_ps[0:5, :, :], AF.Relu, scale=invq[0:5, 3:4])
        nc.scalar.activation(asgT[32:37, :].rearrange("p (h n) -> p h n", h=2),
                             al_ps[32:37, :, :], AF.Relu, scale=invq[32:37, 0:1])

        # ---------- transpose rows 0:37 to [128, 7, 37] ----------
        tp_ps = ps_tp.tile([128, NT, 40], BF16, tag="tp_ps")
        for t in range(NT):
            w = 128 if t < 6 else NP - 6 * 128  # tail 32
            nc.tensor.transpose(tp_ps[0:w, t, 0:37], asgT[0:37, 128 * t:128 * t + w],
                                ident_bf[0:37, 0:37])
        asg_n = sbs.tile([128, NT, 10], BF16, tag="asg_n")
        nc.gpsimd.memset(asg_n[:], 0.0)
        nc.scalar.copy(asg_n[:, 0:6, 0:Q], tp_ps[:, 0:6, 0:Q])
        nc.scalar.copy(asg_n[:, 0:6, Q:10], tp_ps[:, 0:6, 32:37])
        nc.scalar.copy(asg_n[0:32, 6, 0:Q], tp_ps[0:32, 6, 0:Q])
        nc.scalar.copy(asg_n[0:32, 6, Q:10], tp_ps[0:32, 6, 32:37])

        # ---------- masked softmax weights ----------
        e_in = sbs.tile([128, NT, 10], BF16, tag="e_in")
        nc.vector.tensor_tensor(out=e_in[:, :, 0:Q], in0=asg_n[:, :, 0:Q],
                                in1=inv[:, 0:NT].broadcast_to([128, NT, Q]),
                                op=ALU.mult)
        nc.vector.tensor_tensor(out=e_in[:, :, Q:10], in0=asg_n[:, :, Q:10],
                                in1=inv[:, NT:2 * NT].broadcast_to([128, NT, Q]),
                                op=ALU.mult)
        e = sbs.tile([128, NT, 10], BF16, tag="e")
        nc.scalar.activation(e[:], e_in[:], AF.Exp)
        sume = sbs.tile([128, 2 * NT], F32, tag="sume")
        nc.vector.tensor_reduce(sume[:, 0:NT], e[:, :, 0:Q], axis=AX.X, op=ALU.add)
        nc.vector.tensor_reduce(sume[:, NT:2 * NT], e[:, :, Q:10], axis=AX.X,
                                op=ALU.add)
        sumr = sbs.tile([128, 2 * NT], F32, tag="sumr")
        nc.vector.reciprocal(sumr[:], sume[:])
        # stil0 = mask/sume ; stil = stil0*inv
        stil0 = sbs.tile([128, 2 * NT], F32, tag="stil0")
        nc.vector.tensor_tensor(out=stil0[:].rearrange("p (s t) -> p s t", s=2),
                                in0=sumr[:].rearrange("p (s t) -> p s t", s=2),
                                in1=U[:, b, :, :], op=ALU.mult)
        stil = sbs.tile([128, 2 * NT], F32, tag="stil")
        nc.vector.tensor_tensor(out=stil[:], in0=stil0[:], in1=inv[:], op=ALU.mult)

        # wt columns per side: [wg*inv x5 | inv | wg x5] (11 wide)
        wt = sbs.tile([128, NT, 24], BF16, tag="wt")
        nc.vector.tensor_tensor(out=wt[:, :, 0:Q], in0=e[:, :, 0:Q],
                                in1=stil[:, 0:NT].broadcast_to([128, NT, Q]),
                                op=ALU.mult)
        nc.vector.tensor_tensor(out=wt[:, :, 6:6 + Q], in0=e[:, :, 0:Q],
                                in1=stil0[:, 0:NT].broadcast_to([128, NT, Q]),
                                op=ALU.mult)
        nc.vector.tensor_tensor(out=wt[:, :, 12:12 + Q], in0=e[:, :, Q:10],
                                in1=stil[:, NT:2 * NT].broadcast_to([128, NT, Q]),
                                op=ALU.mult)
        nc.vector.tensor_tensor(out=wt[:, :, 18:18 + Q], in0=e[:, :, Q:10],
                                in1=stil0[:, NT:2 * NT].broadcast_to([128, NT, Q]),
                                op=ALU.mult)
        nc.vector.tensor_copy(wt[:, :, Q:6], inv[:, 0:NT, None])
        nc.vector.tensor_scalar_mul(wt[:, :, 17:18], inv[:, NT:2 * NT, None],
                                    1.0 / N)

        # ---------- P / R ----------
        p_ps = ps_p.tile([16, C + 1], F32, tag="p_ps")
        r_ps = ps_r.tile([16, C + 1], F32, tag="r_ps")
        for t in range(NT):
            nc.tensor.matmul(p_ps[0:11, :], wt[:, t, 0:11], ng[:, t, :],
                             start=(t == 0), stop=(t == NT - 1))
        for t in range(NT):
            nc.tensor.matmul(r_ps[0:11, :], wt[:, t, 12:23], nl[:, t, :],
                             start=(t == 0), stop=(t == NT - 1))
        nc.vector.tensor_copy(out_sb[0:11, 0:C + 1], p_ps[0:11, :])
        nc.vector.tensor_copy(out_sb[0:11, C + 1:2 * C + 2], r_ps[0:11, :])

        nc.sync.dma_start(out[b], out_sb[:])


def _neg_index():
    n2 = 2 * Q
    mask = np.ones((n2, n2), dtype=bool)
    np.fill_diagonal(mask, False)
    for i in range(Q):
        mask[i, Q + i] = False
        mask[Q + i, i] = False
    return np.stack([np.where(mask[r])[0] for r in range(n2)])


def _combine(results):
    T1 = 0.0
    G = 0.0
    alphas = []
    betas = []
    vs = []
    sims = []
    for r in results:
        o = np.asarray(r["out"], dtype=np.float64)  # [BL, 16, 780]
        P = o[:, 0:11, 0:C + 1]
        R = o[:, 0:11, C + 1:2 * C + 2]
        sims.append(o[:, 0:10, 770:780])
        Pq, beta, sgc = P[:, 0:Q, 0:C], P[:, 6:6 + Q, C], P[:, Q, 0:C]
        Rq, v, slc = R[:, 0:Q, 0:C], R[:, 6:6 + Q, C], R[:, Q, 0:C]
        T1 += (Pq * Rq).sum()
        G += (sgc * slc).sum()
        alphas.append(np.einsum("bqc,bc->bq", Pq, slc))
        betas.append(beta)
        vs.append(v)
    alpha = np.concatenate(alphas, 0)
    beta = np.concatenate(betas, 0)
    v = np.concatenate(vs, 0)
    g = G / (B * N)
    T2 = ((alpha + (0.1 - g) * beta) * v).sum()
    loss1 = -0.15 * (T1 - T2)

    # query CE from raw gram matrices
    sim = np.concatenate(sims, 0)  # [B, 10, 10]
    d = np.einsum("bii->bi", sim)
    iq = 1.0 / np.maximum(np.sqrt(d), 1e-10)
    sh = sim * iq[:, :, None] * iq[:, None, :]
    rows = np.arange(2 * Q)
    pos = sh[:, rows, (rows + Q) % (2 * Q)]          # [B, 10]
    negs = sh[:, rows[:, None], _NEG_IDX]            # [B, 10, 8]
    logits = np.concatenate([pos[:, :, None], negs], axis=-1)
    m = logits.max(-1)
    ce = m + np.log(np.exp(logits - m[:, :, None]).sum(-1)) - pos
    loss2 = ce.mean()
    return np.float32(loss1 + loss2)


_NEG_IDX = _neg_index()


def _prep(gc, lc, q0, q1, att):
    """Build per-core input maps (host-side sharding + layout)."""
    # natural bf16 with ones column: [B, 128, 7, 385]
    def nat_pack(x):
        pad = np.zeros((B, NT * 128, C + 1), BF)
        pad[:, :N, :C] = x.astype(BF)
        pad[:, :, C] = 1.0
        return np.ascontiguousarray(
            pad.reshape(B, NT, 128, C + 1).transpose(0, 2, 1, 3))

    # transposed fp8: [B, 128, 3, 800]
    def tra_pack(x):
        t8 = np.zeros((B, C, NP), F8)
        t8[:, :, :N] = np.swapaxes(x, 1, 2).astype(F8)
        return np.ascontiguousarray(t8.reshape(B, NK, 128, NP).transpose(0, 2, 1, 3))

    z = np.concatenate([q0, q1], axis=1)  # [B, 10, 384]
    zt = np.zeros((B, C, 16), np.float32)
    zt[:, :, 0:10] = np.swapaxes(z, 1, 2)
    zt = zt.reshape(B, NK, 128, 16).transpose(0, 2, 1, 3)
    ztq_a = np.ascontiguousarray(zt.astype(F8))
    ztb_a = np.ascontiguousarray(zt.astype(BF))

    natg_a = nat_pack(gc)
    natl_a = nat_pack(lc)
    trag_a = tra_pack(gc)
    tral_a = tra_pack(lc)

    # mask U: [core][128, BL, 2, 7]
    af = att.astype(np.float32)  # [128, 784]
    Uf = np.zeros((2, B, NT, 128), np.float32)
    Uf[0, :, :6, :] = af[:B, :768].reshape(B, 6, 128)
    Uf[0, :, 6, :16] = af[:B, 768:]
    Uf[1, :, :6, :] = af[B:, :768].reshape(B, 6, 128)
    Uf[1, :, 6, :16] = af[B:, 768:]
    Uf = Uf.transpose(3, 1, 0, 2)  # [128, B, 2, 7]

    in_maps = []
    for i in range(NCORES):
        s = slice(i * BL, (i + 1) * BL)
        in_maps.append({
            "natg": natg_a[s], "natl": natl_a[s],
            "trag": trag_a[s], "tral": tral_a[s],
            "ztq": ztq_a[s], "ztb": ztb_a[s],
            "u_in": np.ascontiguousarray(Uf[:, s]),
        })
    return in_maps


def kernel(all_queries_0, all_queries_1, gc_output, lc_output, attn_hard,
           gc_spatial_res=None, lc_spatial_res=None):
    if "nc" not in _CACHED:
        _CACHED["nc"] = _build()
    nc = _CACHED["nc"]

    gc = np.asarray(gc_output, dtype=np.float32)
    lc = np.asarray(lc_output, dtype=np.float32)[:, 0]
    q0 = np.asarray(all_queries_0, dtype=np.float32)
    q1 = np.asarray(all_queries_1, dtype=np.float32)
    att = np.asarray(attn_hard, dtype=np.int32).reshape(2 * B, N)

    in_maps = _prep(gc, lc, q0, q1, att)
    res = run_bass_kernel_spmd(nc, in_maps, core_ids=list(range(NCORES)))
    return _combine(res.results)


# revision 11
# speedup vs baseline: 2.7713x; 1.4173x over previous
"""Trainium2 Bass kernel for nn_AlignCriterion (align loss).

Data-parallel over batch: 8 batches per core, 8 cores. The O(B*N^2*C)
correlation/assignment einsums are algebraically collapsed (see _combine).

Layouts shipped from host per batch:
  natural  [128, 7, 385] bf16   x with a ones column  (P/R moving operand)
  transposed [128, 3, 896] fp8  x^T, n padded to 896  (asg moving operand)
  ztq      [128, 3, 16]   fp8   8 * normalized queries^T (asg stationary)
  ztb      [128, 3, 16]   bf16  raw queries^T (CE gram matrix)
  misc     [128, 7, 2, 3] f32   per-row 1/||x||: [inv, inv/8, invR]
  u        [128, 7, 2]    f32   attention masks (t, side)

Device per batch: sim = z z^T; asgT = ztq^T @ xT (both sides into one
PSUM tile, lc at rows 32:37 via tile_position); relu-drain to bf16;
7 combined PE transposes -> [128, 7, (2,5)]; masked softmax weights
wt = [wg*inv | invR | wg] (11 cols/side); P/R matmuls (R at rows 32:43).
The ones column gives beta/v; the inv column gives s_gc / s_lc/784.
Host combines partials in f64. Emission is software-pipelined: batch
b's transposes/PR are emitted after batch b+1's asg matmuls so the PE
stream never stalls on the DVE/ACT softmax round-trip.
"""

import sys

import numpy as np

sys.path.insert(0, "/opt/trn_rl_repo")

import ml_dtypes  # noqa: E402
import concourse.bass as bass  # noqa: E402,F401
import concourse.mybir as mybir  # noqa: E402
import concourse.tile as tile  # noqa: E402
from concourse import bacc  # noqa: E402
from concourse.bass_utils import run_bass_kernel_spmd  # noqa: E402
from concourse.masks import make_identity  # noqa: E402

F32 = mybir.dt.float32
BF16 = mybir.dt.bfloat16
FP8 = mybir.dt.float8e4
AF = mybir.ActivationFunctionType
ALU = mybir.AluOpType
AX = mybir.AxisListType

BF = ml_dtypes.bfloat16
F8 = ml_dtypes.float8_e4m3

B = 64
N = 784          # 28*28 spatial positions
C = 384
Q = 5
NCORES = 8
BL = B // NCORES  # batches per core = 8
NT = 7           # n tiles of 128
NK = 3           # c chunks of 128
NP = 896         # padded n for the transposed layout (7*128)
H = NP // 2      # psum half width (448)

_CACHED = {}


def _build():
    nc = bacc.Bacc("TRN2", target_bir_lowering=False, debug=False,
                   num_devices=NCORES)

    natg = nc.dram_tensor("natg", [BL, 128, NT, C + 1], BF16, kind="ExternalInput").ap()
    natl = nc.dram_tensor("natl", [BL, 128, NT, C + 1], BF16, kind="ExternalInput").ap()
    trag = nc.dram_tensor("trag", [BL, 128, NK, NP], FP8, kind="ExternalInput").ap()
    tral = nc.dram_tensor("tral", [BL, 128, NK, NP], FP8, kind="ExternalInput").ap()
    ztq = nc.dram_tensor("ztq", [BL, 128, NK, 16], FP8, kind="ExternalInput").ap()
    ztb = nc.dram_tensor("ztb", [BL, 128, NK, 16], BF16, kind="ExternalInput").ap()
    misc = nc.dram_tensor("misc", [128, BL, NT, 2, 3], F32, kind="ExternalInput").ap()
    s_in = nc.dram_tensor("s_in", [48, 16], BF16, kind="ExternalInput").ap()
    u_in = nc.dram_tensor("u_in", [128, BL, NT, 2], F32, kind="ExternalInput").ap()
    out = nc.dram_tensor("out", [BL, 48, 400], F32, kind="ExternalOutput").ap()

    with tile.TileContext(nc) as tc:
        _kernel(tc, out, natg, natl, trag, tral, ztq, ztb, misc, u_in, s_in)

    # the installed walrus birverifier rejects EVENT_SEMAPHORE_RANGE_CLEAR
    # (opcode 176, emitted by the Tile kernel-tail sem cleanup). NRT re-inits
    # semaphores per execution, so drop the tail clear entirely.
    for fn in nc.m.functions:
        for blk in fn.blocks:
            il = blk.instructions
            for i in range(len(il) - 1, -1, -1):
                if isinstance(il[i], mybir.InstISA) and il[i].isa_opcode == 176:
                    del il[i]

    nc.compile()
    return nc


def _kernel(tc, out, natg, natl, trag, tral, ztq, ztb, misc, u_in, s_in):
    from contextlib import ExitStack
    with ExitStack() as ctx:
        _kernel_inner(ctx, tc, out, natg, natl, trag, tral, ztq, ztb, misc,
                      u_in, s_in)


def _kernel_inner(ctx, tc, out, natg, natl, trag, tral, ztq, ztb, misc,
                  u_in, s_in):
    nc = tc.nc

    consts = ctx.enter_context(tc.tile_pool(name="consts", bufs=1))
    sbin = ctx.enter_context(tc.tile_pool(name="sbin", bufs=4))
    sbq = ctx.enter_context(tc.tile_pool(name="sbq", bufs=3))
    sbs = ctx.enter_context(tc.tile_pool(name="sbs", bufs=2))
    sbo = ctx.enter_context(tc.tile_pool(name="sbo", bufs=2))
    ps_aa = ctx.enter_context(tc.tile_pool(name="ps_aa", bufs=2, space="PSUM"))
    ps_tp = ctx.enter_context(tc.tile_pool(name="ps_tp", bufs=2, space="PSUM"))
    ps_pr = ctx.enter_context(tc.tile_pool(name="ps_pr", bufs=2, space="PSUM"))

    S = consts.tile([48, 16], BF16, tag="S")
    nc.sync.dma_start(S[:], s_in[:, :])
    U = consts.tile([128, BL, NT, 2], F32, tag="U")
    nc.sync.dma_start(U[:], u_in[:, :, :, :])
    MI = consts.tile([128, BL, NT, 2, 3], F32, tag="MI")
    nc.sync.dma_start(MI[:], misc[:, :, :, :, :])

    st = [None] * BL  # per-batch live tiles for the lagged stage

    def stage_a(b):
        ng = sbin.tile([128, NT, C + 1], BF16, tag="ng")
        nl = sbin.tile([128, NT, C + 1], BF16, tag="nl")
        tg = sbin.tile([128, NK, NP], FP8, tag="tg")
        tl = sbin.tile([128, NK, NP], FP8, tag="tl")
        nc.sync.dma_start(ng[:], natg[b])
        nc.sync.dma_start(nl[:], natl[b])
        nc.sync.dma_start(tg[:], trag[b])
        nc.sync.dma_start(tl[:], tral[b])
        zq = sbq.tile([128, NK, 16], FP8, tag="zq")
        zb = sbq.tile([128, NK, 16], BF16, tag="zb")
        nc.sync.dma_start(zq[:], ztq[b])
        nc.sync.dma_start(zb[:], ztb[b])

        out_sb = sbo.tile([48, 400], F32, tag="out_sb")
        nc.gpsimd.memset(out_sb[:], 0.0)
        asgT = sbs.tile([48, NP], BF16, tag="asgT")
        nc.gpsimd.memset(asgT[:], 0.0)

        # sim (CE gram) + assignment logits share one 2-bank psum tile
        aa_ps = ps_aa.tile([48, 2, 512], F32, tag="aa_ps")
        for k in range(NK):
            nc.tensor.matmul(aa_ps[0:10, 0, 448:458], zb[:, k, 0:10],
                             zb[:, k, 0:10],
                             start=(k == 0), stop=(k == NK - 1))
        for h in range(2):
            for k in range(NK):
                nc.tensor.matmul(aa_ps[0:5, h, 0:H], zq[:, k, 0:Q],
                                 tg[:, k, H * h:H * (h + 1)],
                                 start=(k == 0), stop=(k == NK - 1))
        for h in range(2):
            for k in range(NK):
                nc.tensor.matmul(aa_ps[32:37, h, 0:H], zq[:, k, Q:10],
                                 tl[:, k, H * h:H * (h + 1)],
                                 start=(k == 0), stop=(k == NK - 1),
                                 tile_position=(0, 32))

        # relu + drain to bf16 (DVE), sim drain (ACT)
        nc.vector.tensor_scalar_max(
            asgT[0:5, :].rearrange("p (h n) -> p h n", h=2),
            aa_ps[0:5, :, 0:H], 0.0)
        nc.vector.tensor_scalar_max(
            asgT[32:37, :].rearrange("p (h n) -> p h n", h=2),
            aa_ps[32:37, :, 0:H], 0.0)
        nc.scalar.copy(out_sb[0:10, 385:395], aa_ps[0:10, 0, 448:458])
        st[b] = (ng, nl, asgT, out_sb)

    def stage_b(b):
        ng, nl, asgT, out_sb = st[b]
        st[b] = None

        # selector-"transpose" rows {0:5, 32:37} -> [128, 7, 10]:
        # plain matmul out = asgT.T @ S (stationary = asgT chunk)
        tp_ps = ps_tp.tile([128, NT, 10], F32, tag="tp_ps")
        for t in range(NT):
            nc.tensor.matmul(tp_ps[:, t, :],
                             asgT[0:37, 128 * t:128 * (t + 1)],
                             S[0:37, 0:10], start=True, stop=True)

        inv_ts = MI[:, b, :, :, 0]
        inv8_ts = MI[:, b, :, :, 1]
        invR_ts = MI[:, b, :, :, 2]

        # e = exp(asg * inv/8) ; tp cols: gc 0:5, lc 32:37
        e_in = sbs.tile([128, NT, 10], BF16, tag="e_in")
        nc.vector.tensor_tensor(
            out=e_in[:].rearrange("p t (s q) -> p t s q", s=2),
            in0=tp_ps[:].rearrange("p t (s q) -> p t s q", s=2),
            in1=inv8_ts.broadcast_to([128, NT, 2, Q]), op=ALU.mult)
        e = sbs.tile([128, NT, 10], BF16, tag="e")
        nc.scalar.activation(e[:], e_in[:], AF.Exp)

        sume = sbs.tile([128, NT, 2], F32, tag="sume")
        nc.vector.tensor_reduce(
            sume[:], e[:].rearrange("p t (s q) -> p t s q", s=2),
            axis=AX.X, op=ALU.add)
        sumr = sbs.tile([128, NT, 2], F32, tag="sumr")
        nc.vector.reciprocal(sumr[:], sume[:])
        stil0 = sbs.tile([128, NT, 2], F32, tag="stil0")
        nc.vector.tensor_tensor(out=stil0[:], in0=sumr[:], in1=U[:, b, :, :],
                                op=ALU.mult)
        stil = sbs.tile([128, NT, 2], F32, tag="stil")
        nc.vector.tensor_tensor(out=stil[:], in0=stil0[:], in1=inv_ts,
                                op=ALU.mult)

        # wt columns per side: [wg*inv x5 | invR | wg x5 | pad]
        wt = sbs.tile([128, NT, 24], BF16, tag="wt")
        wt4 = wt[:].rearrange("p t (s c) -> p t s c", s=2)
        e4 = e[:].rearrange("p t (s q) -> p t s q", s=2)
        nc.vector.tensor_tensor(out=wt4[:, :, :, 0:Q], in0=e4[:],
                                in1=stil[:].broadcast_to([128, NT, 2, Q]),
                                op=ALU.mult)
        nc.vector.tensor_tensor(out=wt4[:, :, :, 6:6 + Q], in0=e4[:],
                                in1=stil0[:].broadcast_to([128, NT, 2, Q]),
                                op=ALU.mult)
        nc.vector.tensor_copy(wt4[:, :, :, Q:6],
                              invR_ts.broadcast_to([128, NT, 2, 1]))

        # P rows 0:11, R rows 32:43
        pr_ps = ps_pr.tile([48, C + 1], F32, tag="pr_ps")
        for t in range(NT):
            nc.tensor.matmul(pr_ps[0:11, :], wt[:, t, 0:11], ng[:, t, :],
                             start=(t == 0), stop=(t == NT - 1))
        for t in range(NT):
            nc.tensor.matmul(pr_ps[32:43, :], wt[:, t, 12:23], nl[:, t, :],
                             start=(t == 0), stop=(t == NT - 1),
                             tile_position=(0, 32))
        nc.scalar.copy(out_sb[0:11, 0:C + 1], pr_ps[0:11, :])
        nc.scalar.copy(out_sb[32:43, 0:C + 1], pr_ps[32:43, :])
        nc.gpsimd.dma_start(out[b], out_sb[:])

    for b in range(BL):
        stage_a(b)
        if b > 0:
            stage_b(b - 1)
    stage_b(BL - 1)


def _neg_index():
    n2 = 2 * Q
    mask = np.ones((n2, n2), dtype=bool)
    np.fill_diagonal(mask, False)
    for i in range(Q):
        mask[i, Q + i] = False
        mask[Q + i, i] = False
    return np.stack([np.where(mask[r])[0] for r in range(n2)])


def _combine(results):
    T1 = 0.0
    G = 0.0
    alphas = []
    betas = []
    vs = []
    sims = []
    for r in results:
        o = np.asarray(r["out"], dtype=np.float64)  # [BL, 48, 400]
        P = o[:, 0:11, 0:C + 1]
        R = o[:, 32:43, 0:C + 1]
        sims.append(o[:, 0:10, 385:395])
        Pq, beta, sgc = P[:, 0:Q, 0:C], P[:, 6:6 + Q, C], P[:, Q, 0:C]
        Rq, v, slc = R[:, 0:Q, 0:C], R[:, 6:6 + Q, C], R[:, Q, 0:C]
        T1 += (Pq * Rq).sum()
        G += (sgc * slc).sum()
        alphas.append(np.einsum("bqc,bc->bq", Pq, slc))
        betas.append(beta)
        vs.append(v)
    alpha = np.concatenate(alphas, 0)
    beta = np.concatenate(betas, 0)
    v = np.concatenate(vs, 0)
    g = G / (B * N)
    T2 = ((alpha + (0.1 - g) * beta) * v).sum()
    loss1 = -0.15 * (T1 - T2)

    # query CE from raw gram matrices
    sim = np.concatenate(sims, 0)  # [B, 10, 10]
    d = np.einsum("bii->bi", sim)
    iq = 1.0 / np.maximum(np.sqrt(d), 1e-10)
    sh = sim * iq[:, :, None] * iq[:, None, :]
    rows = np.arange(2 * Q)
    pos = sh[:, rows, (rows + Q) % (2 * Q)]          # [B, 10]
    negs = sh[:, rows[:, None], _NEG_IDX]            # [B, 10, 8]
    logits = np.concatenate([pos[:, :, None], negs], axis=-1)
    m = logits.max(-1)
    ce = m + np.log(np.exp(logits - m[:, :, None]).sum(-1)) - pos
    loss2 = ce.mean()
    return np.float32(loss1 + loss2)


_NEG_IDX = _neg_index()


def _prep(gc, lc, q0, q1, att):
    """Build per-core input maps (host-side sharding + layout)."""
    # natural bf16 with ones column: [B, 128, 7, 385]
    def nat_pack(x):
        pad = np.zeros((B, NT * 128, C + 1), BF)
        pad[:, :N, :C] = x.astype(BF)
        pad[:, :, C] = 1.0
        return np.ascontiguousarray(
            pad.reshape(B, NT, 128, C + 1).transpose(0, 2, 1, 3))

    # transposed fp8: [B, 128, 3, 896]
    def tra_pack(x):
        t8 = np.zeros((B, C, NP), F8)
        t8[:, :, :N] = np.swapaxes(x, 1, 2).astype(F8)
        return np.ascontiguousarray(t8.reshape(B, NK, 128, NP).transpose(0, 2, 1, 3))

    z = np.concatenate([q0, q1], axis=1)  # [B, 10, 384]
    qn = np.linalg.norm(z, axis=-1)       # [B, 10]
    zhat8 = 8.0 * z / np.maximum(qn, 1e-10)[:, :, None]
    def z_pack(zv, dt):
        zt = np.zeros((B, C, 16), np.float32)
        zt[:, :, 0:10] = np.swapaxes(zv, 1, 2)
        return np.ascontiguousarray(
            zt.reshape(B, NK, 128, 16).transpose(0, 2, 1, 3).astype(dt))
    ztq_a = z_pack(zhat8, F8)
    ztb_a = z_pack(z, BF)

    natg_a = nat_pack(gc)
    natl_a = nat_pack(lc)
    trag_a = tra_pack(gc)
    tral_a = tra_pack(lc)

    # row inverse norms on host: inv [2, B, 7, 128] (s: 0=gc, 1=lc)
    nrm = np.stack([np.linalg.norm(gc, axis=-1), np.linalg.norm(lc, axis=-1)])
    invf = np.zeros((2, B, NT * 128), np.float32)
    invf[:, :, :N] = 1.0 / np.maximum(nrm, 1e-10)
    invf[:, :, N:] = 1e10
    invf = invf.reshape(2, B, NT, 128)
    mi = np.zeros((128, B, NT, 2, 3), np.float32)
    mi[:, :, :, :, 0] = invf.transpose(3, 1, 2, 0)
    mi[:, :, :, :, 1] = mi[:, :, :, :, 0] / 8.0
    mi[:, :, :, 0, 2] = mi[:, :, :, 0, 0]
    mi[:, :, :, 1, 2] = mi[:, :, :, 1, 0] / N

    S_sel = np.zeros((48, 16), BF)
    S_sel[np.arange(5), np.arange(5)] = 1
    S_sel[np.arange(32, 37), np.arange(5, 10)] = 1

    # mask U: [128, B, 7, 2]
    af = att.astype(np.float32)  # [128, 784]
    Uf = np.zeros((2, B, NT, 128), np.float32)
    Uf[0, :, :6, :] = af[:B, :768].reshape(B, 6, 128)
    Uf[0, :, 6, :16] = af[:B, 768:]
    Uf[1, :, :6, :] = af[B:, :768].reshape(B, 6, 128)
    Uf[1, :, 6, :16] = af[B:, 768:]
    Uf = Uf.transpose(3, 1, 2, 0)  # [128, B, 7, 2]

    in_maps = []
    for i in range(NCORES):
        s = slice(i * BL, (i + 1) * BL)
        in_maps.append({
            "natg": natg_a[s], "natl": natl_a[s],
            "trag": trag_a[s], "tral": tral_a[s],
            "ztq": ztq_a[s], "ztb": ztb_a[s],
            "misc": np.ascontiguousarray(mi[:, s]),
            "s_in": S_sel,
            "u_in": np.ascontiguousarray(Uf[:, s]),
        })
    return in_maps


def kernel(all_queries_0, all_queries_1, gc_output, lc_output, attn_hard,
           gc_spatial_res=None, lc_spatial_res=None):
    if "nc" not in _CACHED:
        _CACHED["nc"] = _build()
    nc = _CACHED["nc"]

    gc = np.asarray(gc_output, dtype=np.float32)
    lc = np.asarray(lc_output, dtype=np.float32)[:, 0]
    q0 = np.asarray(all_queries_0, dtype=np.float32)
    q1 = np.asarray(all_queries_1, dtype=np.float32)
    att = np.asarray(attn_hard, dtype=np.int32).reshape(2 * B, N)

    in_maps = _prep(gc, lc, q0, q1, att)
    res = run_bass_kernel_spmd(nc, in_maps, core_ids=list(range(NCORES)))
    return _combine(res.results)
